# revision 16
# baseline (speedup 1.0000x reference)
"""Bass/Trainium2 kernel for nn_CD_49555332661898 (RKD-style per-class HuberDist).

Math (per class c, with mask m = targets[:, c] in {0,1}):
  d2 = pairwise sq-distances of rows; ds = sqrt(d2) with zero diag;
  mean over selected off-diag pairs; dsn = ds/mean_s, dtn = dt/mean_t;
  loss_c = sum(huber(dsn - dtn) * M) / n^2, summed over classes with n > 1.

Key identity used on device: with masked features g = m*f laid out [L, N]
(transposed) and two augmentation rows (m, -m*(sq+SLACK)/2), a single PE
accumulation computes  P[i,j] = -(m_i m_j (d2_ij + 2*SLACK))/2  directly, so
  ds~ = sqrt(-2*P + EPS) = m_i m_j ds_ij  (+sqrt(EPS) leakage on masked
pairs and a +SLACK inflation that cancels through the mean normalization),
and all downstream elementwise work needs no masks at all.  Features ride in
bf16 (halves DMA + fast PE weight loads); the aug rows stay f32r so the
sq_i+sq_j terms keep 12-bit-mantissa precision; PSUM accumulates fp32.

Symmetry: d-matrices are symmetric, so row-block r only computes columns
[128r, 512) — i.e. its 128-wide diagonal window (always the FIRST 128
columns of its PSUM slice) plus the strict upper blocks.  Full-matrix sums
are recovered as diag + 2*upper.  The diagonal windows of all four blocks
and each upper column-band are processed by single batched multi-dim ACT
ops, so the per-op overhead does not eat the symmetry savings.

huber == 0.5*u^2 exactly while |u| < 1 everywhere; u = dsn - dtn has
|u| ~ 0.05 for randn inputs, so the kernel computes 0.5*sum(u^2) and returns
per-partition max/min of u so the host can verify the bound held.

Sharding: classes split 10-per-core across 8 NeuronCores (embarrassingly
parallel); each core returns per-class accumulators and the host does the
final tiny reduction.
"""

import os
import sys

import numpy as np

for _p in ("/opt/trn_rl_repo", "/root/.axon_site/_ro/trn_rl_repo"):
    if os.path.isdir(_p) and _p not in sys.path:
        sys.path.insert(0, _p)

import concourse.bacc as bacc
import concourse.tile as tile
import concourse.mybir as mybir
from concourse import bass_isa
from concourse.bass_utils import run_bass_kernel_spmd

F32 = mybir.dt.float32
F32R = mybir.dt.float32r
BF16 = mybir.dt.bfloat16
F16 = mybir.dt.float16
AOP = mybir.AluOpType
AFT = mybir.ActivationFunctionType
AX = mybir.AxisListType

N, C, L = 512, 80, 512
NCORES = 8
CPC = C // NCORES          # classes per core
NB = 4                     # 128-row blocks per [512, 512] matrix
EPS = 1e-8
SLACK = 0.25               # keeps the (rounding-noisy) diagonal of d2 positive

# ACC column layout (per class, [128, 16] fp32)
QD = 0      # 0     sum u^2 over all diag windows (weight 1)
QU = 1      # 1:4   sum u^2 over upper bands (weight 2)
MXC = 4     # 4:8   max u' per block
MNC = 8     # 8:12  min u' per block
ALC = 12    # 12    alpha_s

# RS column layout ([128, 8] fp32): s: diag 0, bands 1:4; t: diag 4, bands 5:8


def _round_fp32r(a: np.ndarray) -> np.ndarray:
    """Round fp32 to the fp32r grid (11-bit mantissa, RTNE, low 12 bits 0)."""
    u = np.ascontiguousarray(a, dtype=np.float32).view(np.uint32)
    bias = np.uint32(0x7FF) + ((u >> np.uint32(12)) & np.uint32(1))
    return ((u + bias) & np.uint32(0xFFFFF000)).view(np.float32)


def _build_program(repeat: int = 1):
    nc = bacc.Bacc("TRN2", target_bir_lowering=False, debug=False,
                   num_devices=NCORES)

    fs_in = nc.dram_tensor("fs", [CPC, 128, NB, 512], BF16,
                           kind="ExternalInput").ap()
    ft_in = nc.dram_tensor("ft", [CPC, 128, NB, 512], BF16,
                           kind="ExternalInput").ap()
    augl_s = nc.dram_tensor("augl_s", [2, CPC, 512], F32R,
                            kind="ExternalInput").ap()
    augr_s = nc.dram_tensor("augr_s", [2, CPC, 512], F32R,
                            kind="ExternalInput").ap()
    augl_t = nc.dram_tensor("augl_t", [2, CPC, 512], F32R,
                            kind="ExternalInput").ap()
    augr_t = nc.dram_tensor("augr_t", [2, CPC, 512], F32R,
                            kind="ExternalInput").ap()
    consts = nc.dram_tensor("consts", [128, 32], F32,
                            kind="ExternalInput").ap()
    acc_out = nc.dram_tensor("acc_out", [CPC, 128, 16], F32,
                             kind="ExternalOutput").ap()

    with tile.TileContext(nc) as tc:
        with tc.tile_pool(name="feat", bufs=3) as featp, \
             tc.tile_pool(name="dmat", bufs=2) as dmatp, \
             tc.tile_pool(name="smol", bufs=2) as smolp, \
             tc.tile_pool(name="stat", bufs=1) as statp, \
             tc.tile_pool(name="psg", bufs=2, space="PSUM") as psg:

            CONSTS = statp.tile([128, 32], F32)
            nc.sync.dma_start(out=CONSTS[:], in_=consts)
            EPST = statp.tile([128, 1], F32)
            nc.vector.memset(EPST[:], EPS)

            import contextlib
            loop_cm = (
                tc.For_i(0, repeat, 1,
                         hint_engines=(mybir.EngineType.PE,
                                       mybir.EngineType.Activation,
                                       mybir.EngineType.DVE,
                                       mybir.EngineType.SP,
                                       mybir.EngineType.Pool))
                if repeat > 1 else contextlib.nullcontext()
            )
            with loop_cm:
                _emit_classes(nc, tc, featp, dmatp, smolp, psg, CONSTS, EPST,
                              fs_in, ft_in, augl_s, augr_s, augl_t, augr_t,
                              acc_out)

    nc.compile()
    return nc


def _emit_classes(nc, tc, featp, dmatp, smolp, psg, CONSTS, EPST,
                  fs_in, ft_in, augl_s, augr_s, augl_t, augr_t, acc_out):
    for c in range(CPC):
        FS = featp.tile([128, NB, 512], BF16, tag="FS")
        nc.sync.dma_start(out=FS[:], in_=fs_in[c])
        FT = featp.tile([128, NB, 512], BF16, tag="FT")
        nc.sync.dma_start(out=FT[:], in_=ft_in[c])
        ALS = smolp.tile([2, 512], F32R, tag="ALS")
        nc.sync.dma_start(out=ALS[:], in_=augl_s[:, c, :])
        ARS = smolp.tile([2, 512], F32R, tag="ARS")
        nc.sync.dma_start(out=ARS[:], in_=augr_s[:, c, :])
        ALT = smolp.tile([2, 512], F32R, tag="ALT")
        nc.sync.dma_start(out=ALT[:], in_=augl_t[:, c, :])
        ART = smolp.tile([2, 512], F32R, tag="ART")
        nc.sync.dma_start(out=ART[:], in_=augr_t[:, c, :])

        DS = dmatp.tile([128, NB, 512], F16, tag="DS")
        DT = dmatp.tile([128, NB, 512], F16, tag="DT")
        RS = smolp.tile([128, 8], F32, tag="RS")

        for (F, AL, AR, D, rd) in (
            (FS, ALS, ARS, DS, 0),
            (FT, ALT, ART, DT, 4),
        ):
            PS = psg.tile([128, NB, 512], F32, tag="PS")
            for r in range(NB):
                W = 512 - 128 * r
                js = slice(128 * r, 128 * r + 128)
                cs = slice(128 * r, 512)
                for kk in range(NB):
                    nc.tensor.matmul(PS[:, r, 0:W], F[:, kk, js],
                                     F[:, kk, cs], start=(kk == 0),
                                     stop=False)
                nc.tensor.matmul(PS[:, r, 0:W], AL[:, js], AR[:, cs],
                                 start=False, stop=True)
            # ds~ = sqrt(m_i m_j (d2+2*SLACK) + EPS); rowsums ride along.
            # Batched: all 4 diag windows in one op (each block's diagonal
            # window is the first 128 columns of its PSUM slice), then one
            # op per upper column-band, so totals can be weighted 1x/2x.
            nc.scalar.activation(D[:, :, 0:128], PS[:, :, 0:128], AFT.Sqrt,
                                 bias=EPST[:], scale=-2.0,
                                 accum_out=RS[:, rd:rd + 1])
            for j in range(1, NB):
                nc.scalar.activation(D[:, 0:NB - j, 128 * j:128 * (j + 1)],
                                     PS[:, 0:NB - j, 128 * j:128 * (j + 1)],
                                     AFT.Sqrt, bias=EPST[:], scale=-2.0,
                                     accum_out=RS[:, rd + j:rd + j + 1])

        # weighted total sums -> means -> alphas (bcast on all partitions)
        RSUM = smolp.tile([128, 8], F32, tag="RSUM")
        nc.gpsimd.partition_all_reduce(RSUM[:], RS[:], 128,
                                       bass_isa.ReduceOp.add)
        VW = smolp.tile([128, 8], F32, tag="VW")
        nc.vector.tensor_tensor(VW[:], RSUM[:], CONSTS[:, 16:24], AOP.mult)
        SUMS = smolp.tile([128, 2], F32, tag="SUMS")
        nc.vector.tensor_reduce(SUMS[:],
                                VW[:].rearrange("p (s b) -> p s b", s=2),
                                AX.X, AOP.add)
        MEANS = smolp.tile([128, 2], F32, tag="MEANS")
        nc.vector.tensor_scalar(MEANS[:], SUMS[:], CONSTS[:, c:c + 1], EPS,
                                AOP.mult, AOP.max)
        ALPH = smolp.tile([128, 2], F32, tag="ALPH")
        nc.vector.reciprocal(ALPH[:], MEANS[:])
        # ratio = alpha_t/alpha_s = mean_s/mean_t
        RATIO = smolp.tile([128, 1], F32, tag="RATIO")
        nc.vector.tensor_tensor(RATIO[:], MEANS[:, 0:1], ALPH[:, 1:2],
                                AOP.mult)

        ACC = smolp.tile([128, 16], F32, tag="ACC")
        nc.vector.tensor_copy(ACC[:, ALC:ALC + 1], ALPH[:, 0:1])
        for r in range(NB):
            W = 512 - 128 * r
            # u' = ds~ - (alpha_t/alpha_s) dt~ ;  u = alpha_s * u'
            nc.vector.tensor_scalar(DT[:, r, 0:W], DT[:, r, 0:W],
                                    RATIO[:], None, AOP.mult)
            nc.vector.tensor_tensor(DT[:, r, 0:W], DS[:, r, 0:W],
                                    DT[:, r, 0:W], AOP.subtract)
            # guard accums: per-partition max and min of u'
            SCR2 = smolp.tile([128, 512], F16, tag="SCR2")
            nc.vector.tensor_scalar(SCR2[:, 0:W], DT[:, r, 0:W], 1.0, None,
                                    AOP.mult, AOP.max,
                                    accum_out=ACC[:, MXC + r:MXC + r + 1])
            SCR3 = smolp.tile([128, 512], F16, tag="SCR3")
            nc.vector.tensor_scalar(SCR3[:, 0:W], DT[:, r, 0:W], 1.0, None,
                                    AOP.mult, AOP.min,
                                    accum_out=ACC[:, MNC + r:MNC + r + 1])
        # sum u^2 = sum (alpha_s u')^2, batched diag + upper bands
        SCR = dmatp.tile([128, NB, 512], F16, tag="SCR")
        nc.scalar.activation(SCR[:, :, 0:128], DT[:, :, 0:128], AFT.Square,
                             scale=ALPH[:, 0:1],
                             accum_out=ACC[:, QD:QD + 1])
        for j in range(1, NB):
            nc.scalar.activation(SCR[:, 0:NB - j, 128 * j:128 * (j + 1)],
                                 DT[:, 0:NB - j, 128 * j:128 * (j + 1)],
                                 AFT.Square, scale=ALPH[:, 0:1],
                                 accum_out=ACC[:, QU + j - 1:QU + j])
        nc.sync.dma_start(out=acc_out[c], in_=ACC[:])


_CACHED_NC = None


def kernel(le_student: np.ndarray, le_teacher: np.ndarray,
           targets: np.ndarray) -> np.ndarray:
    global _CACHED_NC

    bfnp = mybir.dt.np(BF16)
    m = targets.T.astype(np.float32)                      # [C, N]
    nvec = m.sum(axis=1)                                  # positives per class

    def prep(le):
        # [N, C, L] -> masked, transposed, bf16 [C, L, N]
        g = np.ascontiguousarray(le.transpose(1, 2, 0)).astype(np.float32)
        g *= m[:, None, :]
        gb = g.astype(bfnp)
        sq = (gb.astype(np.float64) ** 2).sum(axis=1)     # [C, N]
        # Snap sq+SLACK to the f32r grid FIRST; then *0.5 and *m are exact,
        # so the device-side aug rows carry no extra rounding (the diagonal
        # of d2 stays >= SLACK instead of going sqrt(negative) -> NaN).
        sq_r = _round_fp32r((sq + SLACK).astype(np.float32))
        sqrow = -0.5 * sq_r * m                           # [C, N] f32r grid
        feats = np.ascontiguousarray(
            gb.reshape(C, NB, 128, N).transpose(0, 2, 1, 3))  # [C,128,NB,N]
        return feats, sqrow

    feats_s, sqrow_s = prep(le_student)
    feats_t, sqrow_t = prep(le_teacher)

    inv_cnt = 1.0 / np.maximum(nvec * (nvec - 1.0), 1.0)  # [C]
    wv = np.where(nvec > 1.0, 1.0 / np.maximum(nvec * nvec, 1.0), 0.0)

    w8 = np.array([1, 2, 2, 2, 1, 2, 2, 2], dtype=np.float32)

    in_maps = []
    for k in range(NCORES):
        cs = slice(k * CPC, (k + 1) * CPC)
        consts = np.zeros((128, 32), dtype=np.float32)
        consts[:, 0:CPC] = inv_cnt[cs][None, :]
        consts[:, 16:24] = w8[None, :]
        in_maps.append({
            "fs": feats_s[cs],
            "ft": feats_t[cs],
            "augl_s": np.ascontiguousarray(
                np.stack([m[cs], sqrow_s[cs]], axis=0)),   # [2, CPC, N]
            "augr_s": np.ascontiguousarray(
                np.stack([sqrow_s[cs], m[cs]], axis=0)),
            "augl_t": np.ascontiguousarray(
                np.stack([m[cs], sqrow_t[cs]], axis=0)),
            "augr_t": np.ascontiguousarray(
                np.stack([sqrow_t[cs], m[cs]], axis=0)),
            "consts": consts,
        })

    if _CACHED_NC is None:
        _CACHED_NC = _build_program()
    nc = _CACHED_NC

    res = run_bass_kernel_spmd(nc, in_maps, core_ids=list(range(NCORES)))
    kernel.last_exec_time_ns = res.exec_time_ns
    kernel.last_in_maps = in_maps

    total = 0.0
    for k in range(NCORES):
        acc = np.asarray(res.results[k]["acc_out"], dtype=np.float64)
        qsum = (acc[:, :, QD].sum(axis=1)
                + 2.0 * acc[:, :, QU:QU + 3].sum(axis=(1, 2)))   # [CPC]
        alpha_s = acc[:, 0, ALC]
        umax = max(np.abs(acc[:, :, MXC:MXC + 4]).max(),
                   np.abs(acc[:, :, MNC:MNC + 4]).max())
        umax_scaled = umax * alpha_s.max()
        # huber == 0.5*u^2 exactly when |u| < 1 everywhere; randn inputs
        # keep |u| ~ 0.05 (>20 sigma of margin to the huber knee).
        if umax_scaled >= 1.0:
            raise AssertionError(
                f"max|dsn-dtn| = {umax_scaled} >= 1: huber shortcut invalid")
        w = wv[k * CPC:(k + 1) * CPC]
        total += float((0.5 * qsum * w).sum())

    return np.float32(total)


kernel.last_exec_time_ns = None
kernel.last_in_maps = None


if __name__ == "__main__":
    rng = np.random.default_rng(0)
    le_s = rng.standard_normal((N, C, L)).astype(np.float32)
    le_t = rng.standard_normal((N, C, L)).astype(np.float32)
    tg = rng.integers(0, 2, size=(N, C)).astype(np.int32)
    out = kernel(le_student=le_s, le_teacher=le_t, targets=tg)
    print("kernel out:", out, "exec_ns:", kernel.last_exec_time_ns)
